# revision 25
# baseline (speedup 1.0000x reference)
"""Trainium2 Bass kernel for nn_GCNII_80178449482260 (2x dense GAT + GCNII).

Row-parallel over nodes N=1024 across 8 cores (128 rows each). Per attention
layer: Wh_rows = x_rows @ [W | W@a1 | W@a2] (u,v fall out as two matmul
columns), AllGather of [Wh | v] row-blocks (merged across heads where
possible, Shared outputs), masked softmax on the row-block, att @ Wh_full,
transposed via XBAR DMA-transpose into the next layer's lhsT layout.

DMA queues are specialized: sync = weight streams / input loads, scalar =
transposes + AG bounce-ins, gpsimd = AG-dependent pulls + collectives.
GCNII tail (dead layer 1 elided) tensor-parallel over fm for w@support.
"""
import os
import sys
import numpy as np

for _p in ("/opt/trn_rl_repo", "/root/.axon_site/_ro/trn_rl_repo"):
    if _p not in sys.path:
        sys.path.insert(0, _p)

import ml_dtypes  # noqa: E402
from concourse import bacc, tile, mybir  # noqa: E402
from concourse.bass_utils import run_bass_kernel_spmd  # noqa: E402

BF16 = mybir.dt.bfloat16
F32 = mybir.dt.float32
AF = mybir.ActivationFunctionType
OP = mybir.AluOpType

N = 1024      # nodes
P = 128       # partitions / rows per core
C = 8         # cores
HID = 512
NC1 = 512
H1, H2 = 5, 2
SEG1 = N + 16     # 1040: [Wh(1024) | v*16]
SEG2 = NC1 + 16   # 528
THETA2 = float(np.log(1.25))   # GCNII layer-2 theta; layer 1 is dead code
SLOPE = 0.25
LRELU = 0.01
RG = [list(range(C))]
_NO_CC = bool(int(os.environ.get("KERNEL_NO_CC", "0")))  # profiling stand-in

_CACHE = {}


def _build(reps=1):
    nc = bacc.Bacc("TRN2", target_bir_lowering=False, debug=False,
                   num_devices=C)
    d = {}
    d["xT_sl"] = nc.dram_tensor("xT_sl", [N, P], BF16, kind="ExternalInput")
    d["adj_r"] = nc.dram_tensor("adj_r", [P, N], F32, kind="ExternalInput")
    d["Wg1a"] = nc.dram_tensor("Wg1a", [H1, N, N + 2], BF16, kind="ExternalInput")
    d["Wo1a"] = nc.dram_tensor("Wo1a", [H1 * N, NC1 + 2], BF16, kind="ExternalInput")
    d["Wg2a"] = nc.dram_tensor("Wg2a", [H2, NC1, NC1 + 2], BF16, kind="ExternalInput")
    d["Wo2a"] = nc.dram_tensor("Wo2a", [N, N + 2], BF16, kind="ExternalInput")
    d["fc0_w"] = nc.dram_tensor("fc0_w", [N, HID], BF16, kind="ExternalInput")
    d["fc0_b"] = nc.dram_tensor("fc0_b", [HID], BF16, kind="ExternalInput")
    d["fc1_w"] = nc.dram_tensor("fc1_w", [HID, N], BF16, kind="ExternalInput")
    d["fc1_b"] = nc.dram_tensor("fc1_b", [N], BF16, kind="ExternalInput")
    d["cw1T_sl"] = nc.dram_tensor("cw1T_sl", [N, P], BF16, kind="ExternalInput")
    out_d = nc.dram_tensor("out", [P, N], F32, kind="ExternalOutput")

    with tile.TileContext(nc) as tc:
        _body(nc, tc, d, out_d, reps)
    nc.compile()
    return nc


def _body(nc, tc, d, out_d, reps=1):
    with (
        tc.tile_pool(name="per", bufs=1) as per,        # cross-phase persistents
        tc.tile_pool(name="whv", bufs=1) as whv_p,      # pre-AG staging
        tc.tile_pool(name="whfp", bufs=2) as whf_p,     # gathered Wh_full tiles
        tc.tile_pool(name="wch", bufs=8) as wch_p,      # weight chunk stream
        tc.tile_pool(name="attp", bufs=2) as att_p,     # attention work
        tc.tile_pool(name="scbf", bufs=2) as sc_bf,     # bf16 scratch
        tc.tile_pool(name="sc32", bufs=2) as sc_32,     # f32 scratch
        tc.tile_pool(name="smv", bufs=2) as sm_vec,     # [1, *] vectors
        tc.tile_pool(name="smt", bufs=4) as sm,         # tiny per-row vecs
        tc.tile_pool(name="pswh", bufs=2, space="PSUM") as ps_wh,
        tc.tile_pool(name="psuv", bufs=1, space="PSUM") as ps_uv,
        tc.tile_pool(name="pso1", bufs=1, space="PSUM") as ps_o1,
        tc.tile_pool(name="dram", bufs=1, space="DRAM") as dram,
    ):
        xT_sb = per.tile([P, C, P], BF16, tag="xtsl")    # x_rows^T, k-chunk c
        nc.sync.dma_start(xT_sb[:], d["xT_sl"].ap().rearrange("(c p) m -> p c m", p=P))
        cw1T_sb = per.tile([P, C, P], BF16, tag="cw1t")
        nc.sync.dma_start(cw1T_sb[:], d["cw1T_sl"].ap().rearrange("(c p) m -> p c m", p=P))

        adj_sb = sc_32.tile([P, N], F32, tag="s32")
        nc.sync.dma_start(adj_sb[:], d["adj_r"].ap())
        madj = per.tile([P, N], BF16, tag="madj")        # 0 where adj>0 else -9e15
        nc.vector.tensor_scalar(madj[:], adj_sb[:], 0.0, None, op0=OP.is_gt)
        nc.vector.tensor_scalar(madj[:], madj[:], 1.0, 9e15,
                                op0=OP.subtract, op1=OP.mult)

        # ---------------- helpers ----------------
        def w_stream(src_3d_ap, L):
            """rhs_fn(c) -> [128, L+2] SBUF AP of chunk c of DRAM [*, nk, L+2]."""
            def fn(c):
                t = wch_p.tile([P, N + 2], BF16, tag="wch")
                nc.sync.dma_start(t[:, :L + 2], src_3d_ap[:, c, :])
                return t[:, :L + 2]
            return fn

        def allgather(src_sb, rows, cols, tag):
            ag_in = dram.tile([rows, cols], BF16, tag=f"agi_{tag}")
            nc.sync.dma_start(ag_in[:], src_sb)
            if _NO_CC:
                ag_out = dram.tile([C * rows, cols], BF16, tag=f"ago_{tag}")
                for cc in range(C):
                    nc.gpsimd.dma_start(ag_out[cc * rows:(cc + 1) * rows, :], ag_in[:])
            else:
                ag_out = dram.tile([C * rows, cols], BF16, tag=f"ago_{tag}",
                                   addr_space="Shared")
                nc.gpsimd.collective_compute(
                    "AllGather", OP.bypass, replica_groups=RG,
                    ins=[ag_in.opt()], outs=[ag_out.opt()])
            return ag_out

        def cast_copy(dst, src):
            nc.vector.tensor_scalar(dst, src, 1.0, None, op0=OP.mult)

        def wh_phase(lhsT_fn, lhs_idx, rhs_fn, L, whv_dst, u_dst):
            """Wh = lhsT.T @ [W|Wa1|Wa2]; writes [Wh|v-pad] bf16 into whv_dst,
            u (f32 [P,1]) into u_dst."""
            nk = len(lhs_idx)
            wh = ps_wh.tile([P, N], F32, tag="whps")
            uvp = ps_uv.tile([P, 2], F32, tag="uv")
            for i, c in enumerate(lhs_idx):
                rhs = rhs_fn(c)
                lhsT = lhsT_fn(c)
                for s in range(0, L, 512):
                    w = min(512, L - s)
                    nc.tensor.matmul(wh[:, s:s + w], lhsT, rhs[:, s:s + w],
                                     start=(i == 0), stop=(i == nk - 1))
                nc.tensor.matmul(uvp[:], lhsT, rhs[:, L:L + 2],
                                 start=(i == 0), stop=(i == nk - 1))
            # PSUM evacuation on ACT (DVE is the attention-phase bottleneck)
            nc.scalar.activation(whv_dst[:, :L], wh[:, :L], AF.Copy)
            nc.vector.tensor_copy(u_dst, uvp[:, 0:1])
            nc.vector.tensor_scalar(whv_dst[:, L:L + 16],
                                    uvp[:, 1:2].to_broadcast((P, 16)),
                                    1.0, None, op0=OP.mult)

        def score_phase(ago, seg_off, seg_w, L, u_ap):
            """Masked-softmax numerator n^T-ready state for one head."""
            ago3 = ago[:].rearrange("(c p) f -> p c f", p=P)
            wh_full = whf_p.tile([P, C, SEG1], BF16, tag="whfull")
            for j0 in range(0, C, 4):
                nc.gpsimd.dma_start(wh_full[:, j0:j0 + 4, :seg_w],
                                    ago3[:, j0:j0 + 4, seg_off:seg_off + seg_w])
            v_sb = sm_vec.tile([1, N], BF16, tag="vfull")
            nc.gpsimd.dma_start(
                v_sb[:1, :],
                ago[:, seg_off + L:seg_off + L + 1].rearrange(
                    "(c p) o -> o (c p)", p=P))
            vb = att_p.tile([P, N], BF16, tag="vb")
            nc.gpsimd.partition_broadcast(vb[:], v_sb[:1, :])
            e_sb = sc_32.tile([P, N], F32, tag="s32")
            nc.vector.scalar_tensor_tensor(e_sb[:], vb[:], u_ap, madj[:],
                                           op0=OP.add, op1=OP.add)
            el = sc_32.tile([P, N], F32, tag="s32b")
            nc.vector.scalar_tensor_tensor(el[:], e_sb[:], LRELU, e_sb[:],
                                           op0=OP.mult, op1=OP.max)
            n_bf = att_p.tile([P, N], BF16, tag="nbf")
            ssum = sm.tile([P, 1], F32, tag="ssum")
            nc.scalar.activation(n_bf[:], el[:], AF.Exp, accum_out=ssum[:])
            rs = sm.tile([P, 1], F32, tag="rs")
            nc.vector.reciprocal(rs[:], ssum[:])
            attT = att_p.tile([P, C, P], BF16, tag="attT")
            nc.scalar.dma_start(attT[:], n_bf[:], transpose=True)
            return {"wh": wh_full, "attT": attT, "rs": rs}

        def out_phase(st, L, out_T_ap, o1_hook=None):
            """att@Wh + rs-scaled elu + transposed store for one head."""
            wh_full, attT, rs = st["wh"], st["attT"], st["rs"]
            o_ps = ps_wh.tile([P, N], F32, tag="whps")
            for j in range(C):
                for s in range(0, L, 512):
                    w = min(512, L - s)
                    nc.tensor.matmul(o_ps[:, s:s + w], attT[:, j, :],
                                     wh_full[:, j, s:s + w],
                                     start=(j == 0), stop=(j == C - 1))
            # elu(rs*x) = expm1(min(rs*x,0)) + max(rs*x,0), scale fused
            m_bf = sc_bf.tile([P, N], BF16, tag="elum")
            nc.vector.tensor_scalar(m_bf[:, :L], o_ps[:, :L], rs[:], 0.0,
                                    op0=OP.mult, op1=OP.min)
            r32 = sc_32.tile([P, N], F32, tag="s32b")
            nc.vector.tensor_scalar(r32[:, :L], o_ps[:, :L], rs[:], 0.0,
                                    op0=OP.mult, op1=OP.max)
            g32 = sc_32.tile([P, N], F32, tag="s32")
            nc.scalar.activation(g32[:, :L], m_bf[:, :L], AF.Exp)
            o_bf = sc_bf.tile([P, N], BF16, tag="obf")
            nc.vector.scalar_tensor_tensor(o_bf[:, :L], g32[:, :L], -1.0,
                                           r32[:, :L], op0=OP.add, op1=OP.add)
            nc.sync.dma_start(out_T_ap, o_bf[:, :L], transpose=True)
            if o1_hook is not None:
                o1_hook()

        def att_phase(ago, seg_off, seg_w, L, u_ap, out_T_ap, o1_hook=None):
            st = score_phase(ago, seg_off, seg_w, L, u_ap)
            out_phase(st, L, out_T_ap, o1_hook)

        # per-head tiles: keeps WAR tracking disjoint so heads pipeline
        hcatTs = [per.tile([P, C, P], BF16, tag=f"hcatT{h}", name=f"hcatT{h}")
                  for h in range(H1)]
        xgT = per.tile([P, 4, P], BF16, tag="xgT")
        hcat2Ts = [per.tile([P, 4, P], BF16, tag=f"hcat2T{h}", name=f"hcat2T{h}")
                   for h in range(H2)]
        xg2T = per.tile([P, C, P], BF16, tag="xg2T")
        h0f = per.tile([P, HID], F32, tag="h0f")
        u_t = [per.tile([P, 1], F32, tag=f"u{i}", name=f"u{i}") for i in range(9)]

        fc0_v = d["fc0_w"].ap().rearrange("(c p) f -> p c f", p=P)
        fc1_v = d["fc1_w"].ap().rearrange("(c p) f -> p c f", p=P)

        for _rep in range(reps):
            whv_a = whv_p.tile([P, 2 * SEG1], BF16, tag="whva")
            whv_b = whv_p.tile([P, 3 * SEG1 + HID], BF16, tag="whvb")

            # -------- bulk weight preloads (no slot-recycling waits) --------
            wo1_sb = per.tile([P, H1 * C, NC1 + 2], BF16, tag="wo1")
            nc.sync.dma_start(wo1_sb[:], d["Wo1a"].ap().rearrange(
                "(c p) f -> p c f", p=P))
            wg2_sb = per.tile([P, H2, 4, NC1 + 2], BF16, tag="wg2")
            for h in range(H2):
                nc.sync.dma_start(wg2_sb[:, h], d["Wg2a"].ap()[h].rearrange(
                    "(c p) f -> p c f", p=P))
            wo2_sb = per.tile([P, C, N + 2], BF16, tag="wo2")
            nc.sync.dma_start(wo2_sb[:], d["Wo2a"].ap().rearrange(
                "(c p) f -> p c f", p=P))
            fc1_sb = per.tile([P, 4, N], BF16, tag="fc1")
            nc.sync.dma_start(fc1_sb[:], fc1_v[:])

            # ================= GAT1: 5 heads, 2 merged AGs =================
            def g1_head(h):
                w_fn = w_stream(d["Wg1a"].ap()[h].rearrange("(c p) f -> p c f", p=P), N)
                dst = whv_a[:, h * SEG1:(h + 1) * SEG1] if h < 2 else \
                    whv_b[:, (h - 2) * SEG1:(h - 1) * SEG1]
                wh_phase(lambda c: xT_sb[:, c, :], range(C), w_fn, N,
                         dst, u_t[h][:])

            g1_head(0)

            # -------- GCNII h0 (floats into PE bubbles; rides AG_b) --------
            h0_ps = ps_uv.tile([P, HID], F32, tag="h0ps")
            for c in range(C):
                t = wch_p.tile([P, N + 2], BF16, tag="wch")
                nc.sync.dma_start(t[:, :HID], fc0_v[:, c, :])
                nc.tensor.matmul(h0_ps[:], xT_sb[:, c, :], t[:, :HID],
                                 start=(c == 0), stop=(c == C - 1))
            b_bc = att_p.tile([P, N], BF16, tag="vb")
            nc.sync.dma_start(b_bc[:, :HID],
                              d["fc0_b"].ap()[None, :].to_broadcast((P, HID)))
            nc.vector.scalar_tensor_tensor(h0f[:], h0_ps[:], 1.0,
                                           b_bc[:, :HID], op0=OP.mult, op1=OP.add)
            nc.vector.scalar_tensor_tensor(h0f[:], h0f[:], SLOPE, h0f[:],
                                           op0=OP.mult, op1=OP.max)

            for h in range(1, H1):
                g1_head(h)
            # h0 rides in whv_b tail
            cast_copy(whv_b[:, 3 * SEG1:3 * SEG1 + HID], h0f[:])
            ago_a = allgather(whv_a[:], P, 2 * SEG1, "a")
            ago_b = allgather(whv_b[:], P, 3 * SEG1 + HID, "b")

            # o1 Wh + uv accumulate incrementally as heads complete
            o1_wh = ps_o1.tile([P, NC1], F32, tag="o1wh")
            o1_uv = ps_uv.tile([P, 2], F32, tag="uvo1")

            def mk_o1_hook(h):
                def hook():
                    for j in range(C):
                        k = h * C + j
                        nc.tensor.matmul(o1_wh[:], hcatTs[h][:, j, :],
                                         wo1_sb[:, k, :NC1],
                                         start=(k == 0), stop=(k == H1 * C - 1))
                        nc.tensor.matmul(o1_uv[:], hcatTs[h][:, j, :],
                                         wo1_sb[:, k, NC1:NC1 + 2],
                                         start=(k == 0), stop=(k == H1 * C - 1))
                return hook

            # skew-1 software pipeline: score(h+1) emitted before out(h) so
            # each engine FIFO interleaves heads instead of chain-stepping
            sts = {}

            def g1_score(h):
                ago, off = (ago_a, h * SEG1) if h < 2 else (ago_b, (h - 2) * SEG1)
                sts[h] = score_phase(ago, off, SEG1, N, u_t[h][:])

            def g1_out(h):
                out_phase(sts[h], N, hcatTs[h][:, :, :], o1_hook=mk_o1_hook(h))

            g1_score(0)
            g1_score(1)
            g1_out(0)
            g1_score(2)
            g1_out(1)
            g1_score(3)
            g1_out(2)
            g1_score(4)
            g1_out(3)
            g1_out(4)

            # ================= GAT1 out-attention =================
            whv_o1 = whv_p.tile([P, SEG2], BF16, tag="whvo1")
            nc.scalar.activation(whv_o1[:, :NC1], o1_wh[:], AF.Copy)
            nc.vector.tensor_copy(u_t[5][:], o1_uv[:, 0:1])
            nc.vector.tensor_scalar(whv_o1[:, NC1:NC1 + 16],
                                    o1_uv[:, 1:2].to_broadcast((P, 16)),
                                    1.0, None, op0=OP.mult)
            ago_o1 = allgather(whv_o1[:], P, SEG2, "o1")
            att_phase(ago_o1, 0, SEG2, NC1, u_t[5][:], xgT[:, :, :])

            # ================= GAT2: 2 heads, merged AG =================
            whv_g2 = whv_p.tile([P, 2 * SEG2], BF16, tag="whvg2")
            for h in range(H2):
                wh_phase(lambda c: xgT[:, c, :], range(4),
                         lambda c, hh=h: wg2_sb[:, hh, c, :], NC1,
                         whv_g2[:, h * SEG2:(h + 1) * SEG2], u_t[6 + h][:])
            ago_g2 = allgather(whv_g2[:], P, 2 * SEG2, "g2")
            g2_st = [score_phase(ago_g2, h * SEG2, SEG2, NC1, u_t[6 + h][:])
                     for h in range(H2)]
            for h in range(H2):
                out_phase(g2_st[h], NC1, hcat2Ts[h][:, :, :])

            # ================= GAT2 out-attention =================
            whv_o2 = whv_p.tile([P, SEG1], BF16, tag="whvo2")
            wh_phase(lambda c: hcat2Ts[c // 4][:, c % 4, :], range(C),
                     lambda c: wo2_sb[:, c, :], N,
                     whv_o2[:], u_t[8][:])
            ago_o2 = allgather(whv_o2[:], P, SEG1, "o2")
            att_phase(ago_o2, 0, SEG1, N, u_t[8][:], xg2T[:, :, :])

            # ================= GCNII =================
            h0_full = whf_p.tile([P, C, SEG1], BF16, tag="whfull")
            agb3 = ago_b[:].rearrange("(c p) f -> p c f", p=P)
            nc.gpsimd.dma_start(h0_full[:, :, :HID],
                                agb3[:, :, 3 * SEG1:3 * SEG1 + HID])
            hi_ps = ps_wh.tile([P, N], F32, tag="whps")
            for j in range(C):
                nc.tensor.matmul(hi_ps[:, :HID], xg2T[:, j, :],
                                 h0_full[:, j, :HID],
                                 start=(j == 0), stop=(j == C - 1))
            sf = per.tile([P, HID], F32, tag="sf")
            nc.vector.scalar_tensor_tensor(sf[:], hi_ps[:, :HID], 9.0, h0f[:],
                                           op0=OP.mult, op1=OP.add)
            nc.vector.tensor_scalar(sf[:], sf[:], 0.1, None, op0=OP.mult)
            sb_bf = whv_p.tile([P, HID], BF16, tag="whvs")
            cast_copy(sb_bf[:], sf[:])
            ago_s = allgather(sb_bf[:], P, HID, "s")
            s_full = whf_p.tile([P, C, SEG1], BF16, tag="whfull")
            nc.gpsimd.dma_start(
                s_full[:, :, :HID],
                ago_s[:].rearrange("(c p) f -> p c f", p=P))

            mm_ps = ps_wh.tile([P, N], F32, tag="whps")
            for c in range(C):
                nc.tensor.matmul(mm_ps[:, :HID], cw1T_sb[:, c, :],
                                 s_full[:, c, :HID],
                                 start=(c == 0), stop=(c == C - 1))
            hf = sc_32.tile([P, N], F32, tag="s32")
            nc.vector.scalar_tensor_tensor(hf[:, :HID], sf[:], (1.0 - THETA2) / THETA2,
                                           mm_ps[:, :HID], op0=OP.mult, op1=OP.add)
            nc.vector.scalar_tensor_tensor(hf[:, :HID], hf[:, :HID], THETA2, h0f[:],
                                           op0=OP.mult, op1=OP.add)
            nc.vector.scalar_tensor_tensor(hf[:, :HID], hf[:, :HID], SLOPE, hf[:, :HID],
                                           op0=OP.mult, op1=OP.max)
            hb = sc_bf.tile([P, HID], BF16, tag="hb")
            cast_copy(hb[:], hf[:, :HID])
            hT = per.tile([P, 4, P], BF16, tag="hT")
            nc.scalar.dma_start(hT[:], hb[:], transpose=True)

            y_ps = ps_wh.tile([P, N], F32, tag="whps")
            for c in range(4):
                for fh in range(2):
                    nc.tensor.matmul(y_ps[:, fh * 512:(fh + 1) * 512], hT[:, c, :],
                                     fc1_sb[:, c, fh * 512:(fh + 1) * 512],
                                     start=(c == 0), stop=(c == 3))
            b1_bc = att_p.tile([P, N], BF16, tag="vb")
            nc.sync.dma_start(b1_bc[:],
                              d["fc1_b"].ap()[None, :].to_broadcast((P, N)))
            y_sb = sc_32.tile([P, N], F32, tag="s32")
            nc.vector.scalar_tensor_tensor(y_sb[:], y_ps[:], 1.0, b1_bc[:],
                                           op0=OP.mult, op1=OP.add)
            nc.sync.dma_start(out_d.ap(), y_sb[:])


def _shard_inputs(inputs):
    f32 = lambda a: np.asarray(a, dtype=np.float32)
    bf = lambda a: np.ascontiguousarray(f32(a)).astype(ml_dtypes.bfloat16)
    x = f32(inputs["x"])
    adj = f32(inputs["adj"])
    xT_bf = np.ascontiguousarray(bf(x).T)
    cw1T = np.ascontiguousarray(bf(inputs["cw1"]).T)

    def aug(W, a):
        # [W | W@a1 | W@a2] with fp32 matvecs on host
        W = f32(W)
        Fo = W.shape[1]
        a = f32(a).reshape(-1)
        u_col = W @ a[:Fo]
        v_col = W @ a[Fo:]
        return bf(np.concatenate([W, u_col[:, None], v_col[:, None]], axis=1))

    Wg1a = np.stack([aug(inputs["Wg1"][h], inputs["ag1"][h]) for h in range(H1)])
    Wo1a = aug(inputs["Wo1"], inputs["ao1"])
    Wg2a = np.stack([aug(inputs["Wg2"][h], inputs["ag2"][h]) for h in range(H2)])
    Wo2a = aug(inputs["Wo2"], inputs["ao2"])

    shared = {
        "Wg1a": Wg1a, "Wo1a": Wo1a, "Wg2a": Wg2a, "Wo2a": Wo2a,
        "fc0_w": bf(inputs["fc0_w"]),
        "fc0_b": bf(inputs["fc0_b"]),
        "fc1_w": bf(inputs["fc1_w"]),
        "fc1_b": bf(inputs["fc1_b"]),
    }
    in_maps = []
    for c in range(C):
        r0, r1 = c * P, (c + 1) * P
        m = dict(shared)
        m["xT_sl"] = np.ascontiguousarray(xT_bf[:, r0:r1])
        m["adj_r"] = np.ascontiguousarray(adj[r0:r1])
        m["cw1T_sl"] = np.ascontiguousarray(cw1T[:, r0:r1])
        in_maps.append(m)
    return in_maps


def kernel(**inputs) -> np.ndarray:
    if "nc" not in _CACHE:
        _CACHE["nc"] = _build()
    nc = _CACHE["nc"]
    in_maps = _shard_inputs(inputs)
    res = run_bass_kernel_spmd(nc, in_maps, core_ids=list(range(C)))
    out = np.concatenate([res.results[c]["out"] for c in range(C)], axis=0)
    return np.asarray(out, dtype=np.float32)


if __name__ == "__main__":
    rng = np.random.default_rng(0)
    fake = {
        "x": rng.standard_normal((N, N), dtype=np.float32),
        "adj": np.maximum((rng.random((N, N)) < 0.02).astype(np.float32),
                          np.eye(N, dtype=np.float32)),
        "Wg1": rng.standard_normal((H1, N, N), dtype=np.float32) * 0.02,
        "ag1": rng.standard_normal((H1, 2 * N, 1), dtype=np.float32) * 0.02,
        "Wo1": rng.standard_normal((H1 * N, NC1), dtype=np.float32) * 0.02,
        "ao1": rng.standard_normal((2 * NC1, 1), dtype=np.float32) * 0.02,
        "Wg2": rng.standard_normal((H2, NC1, NC1), dtype=np.float32) * 0.02,
        "ag2": rng.standard_normal((H2, 2 * NC1, 1), dtype=np.float32) * 0.02,
        "Wo2": rng.standard_normal((N, N), dtype=np.float32) * 0.02,
        "ao2": rng.standard_normal((2 * N, 1), dtype=np.float32) * 0.02,
        "fc0_w": rng.standard_normal((N, HID), dtype=np.float32) * 0.02,
        "fc0_b": np.zeros(HID, np.float32),
        "fc1_w": rng.standard_normal((HID, N), dtype=np.float32) * 0.02,
        "fc1_b": np.zeros(N, np.float32),
        "cw0": rng.standard_normal((N, N), dtype=np.float32),
        "cw1": rng.standard_normal((N, N), dtype=np.float32),
    }
    y = kernel(**fake)
    print("kernel ran, out shape", y.shape, "finite:", np.isfinite(y).all())
